# revision 15
# baseline (speedup 1.0000x reference)
"""Trainium2 Bass kernel for an AttentionBlock (GroupNorm -> QKV 1x1 -> full
softmax attention over H*W tokens -> proj 1x1 -> residual).

Sharding: 8 cores = 4 batches x 2 query-halves, no collectives. Per core,
tokens are ordered [own half | other half]; attention is permutation-
invariant over keys, so K/V built in that order need no reshuffling.

Compute strategy (v2):
- fp8e4 DoubleRow matmuls (0.5 cyc/row) for K/V/Q/QK/PV; bf16 proj.
- Scores are computed TRANSPOSED (S^T[m,n] = sum_c K[c,m] Q[c,n]) so the
  exp() output is already in [key, query] layout and feeds the PV matmul
  directly -- no PE transposes at all. Row-sums (denominator) come from a
  ones-column DoubleRow matmul accumulated alongside PV.
- Weights are scaled by 16 host-side to center them in fp8e4 range; the
  resulting 16x factors cancel in softmax normalization (ones value = 16)
  and the 1/sqrt(C) score scale is folded into the exp() activation.
- x (both halves), Q, K, V all stay resident in SBUF: HBM traffic is just
  x in (8MB) + weights (~1.3MB) + out (4MB).
- Dummy bf16 matmuls keep the PE HAM-warm during the x-DMA/GN-stats
  preamble so real matmuls start at 2.4 GHz.

Self-contained: hardcodes shapes from the problem spec
(x: [4, 512, 64, 64] fp32).
"""

import sys

if "/opt/trn_rl_repo" not in sys.path:
    sys.path.insert(0, "/opt/trn_rl_repo")

from contextlib import ExitStack

import numpy as np
import ml_dtypes

import concourse.bass as bass
import concourse.tile as tile
from concourse import mybir
from concourse.bass_utils import run_bass_kernel_spmd

# Problem constants
B = 4
C = 512
H = 64
W = 64
N = H * W          # 4096 tokens
G = 8              # groupnorm groups
EPS = 1e-5
NCORES = 8
NQ = N // 2        # queries per core
P = 128
CT = C // P        # 4 channel tiles
NT = N // P        # 32 key tiles
CHUNK = 512        # n-chunk granularity
NCH = NQ // CHUNK  # 4 chunks per half
NG = NQ // CHUNK   # 4 query groups per core

WS = 16.0          # host-side fp8 weight scale
OFF = 2.5          # exp offset (S max is ~6.0 for this input)
EXP_SCALE = 1.0 / (np.sqrt(np.float32(C)) * WS * WS)

NWARM = 64         # PE warmup dummies during preamble

F32 = mybir.dt.float32
BF16 = mybir.dt.bfloat16
F8 = mybir.dt.float8e4
AF = mybir.ActivationFunctionType
DR = mybir.MatmulPerfMode.DoubleRow

MAX_WAITS_PER_INST = 1  # this walrus drop rejects >1 sync wait per inst


def split_multi_waits(nc: bass.Bass):
    """Walrus codegen here accepts at most one sync wait per instruction.
    Move excess waits onto freshly inserted same-engine NoOps directly
    before the offending instruction (waits just fire earlier)."""
    k = 0
    for fn in nc.m.functions:
        for bb in fn.blocks:
            insts = bb.instructions
            out = []
            changed = False
            for ins in insts:
                si = ins.sync_info
                if si is not None and len(si.on_wait) > MAX_WAITS_PER_INST:
                    waits = list(si.on_wait)
                    keep = waits[-MAX_WAITS_PER_INST:]
                    extra = waits[:-MAX_WAITS_PER_INST]
                    for i in range(0, len(extra), MAX_WAITS_PER_INST):
                        nop = mybir.InstNoOp(
                            name=f"{ins.name}_sw{k}", ins=[], outs=[]
                        )
                        k += 1
                        nop.engine = ins.engine
                        nop.sync_info = mybir.SyncInfo(
                            on_wait=extra[i:i + MAX_WAITS_PER_INST],
                            on_update=[],
                        )
                        out.append(nop)
                    ins.sync_info = mybir.SyncInfo(
                        on_wait=keep, on_update=list(si.on_update)
                    )
                    changed = True
                out.append(ins)
            if changed:
                bb.instructions = out


def build_program(has_bq: bool, has_bp: bool) -> bass.Bass:
    nc = bass.Bass()

    x8_p = nc.declare_dram_parameter("x8", [C, N], BF16, isOutput=False)
    x_a = nc.declare_dram_parameter("x_a", [C, NQ], F32, isOutput=False)
    wq_p = nc.declare_dram_parameter("wq8", [C, C], F8, isOutput=False)
    wk_p = nc.declare_dram_parameter("wk8", [C, C], F8, isOutput=False)
    wv_p = nc.declare_dram_parameter("wv8", [C, C], F8, isOutput=False)
    wp_p = nc.declare_dram_parameter("wp_bf", [C, C], BF16, isOutput=False)
    bq_p = nc.declare_dram_parameter("bq16", [C], F32, isOutput=False)
    bp_p = nc.declare_dram_parameter("bp", [C], F32, isOutput=False)
    gnw_p = nc.declare_dram_parameter("gn_w", [C], F32, isOutput=False)
    gnb_p = nc.declare_dram_parameter("gn_b", [C], F32, isOutput=False)
    out_q = nc.declare_dram_parameter("out_q", [C, NQ], F32, isOutput=True)

    # channel layout everywhere: c = ct*128 + p  (partition-inner)
    x8r = x8_p[:].rearrange("(ct p) n -> p ct n", p=P)
    xar = x_a[:].rearrange("(ct p) n -> p ct n", p=P)
    outr = out_q[:].rearrange("(ct p) n -> p ct n", p=P)

    with tile.TileContext(nc) as tc, ExitStack() as ctx:
        big = ctx.enter_context(tc.tile_pool(name="big", bufs=1))
        const = ctx.enter_context(tc.tile_pool(name="const", bufs=1))

        K_sb = big.tile([P, CT, N], F8)       # K^T: [c, m], 16x scaled
        vT_sb = big.tile([P, NT, C], F8)      # V: [m, c], 16x scaled
        q_sb = big.tile([P, CT, NQ], F8)      # Q^T: [c, n], 16x scaled
        x8_sb = big.tile([P, CT, N], BF16)    # bf16 x, both halves (stats/GN)
        xa_sb = big.tile([P, CT, NQ], F32)    # own half fp32 (residual only)

        # constants / weights: tiny DMAs first, then x, then big weights
        gnw_sb = const.tile([P, CT], F32)
        nc.sync.dma_start(gnw_sb, gnw_p[:].rearrange("(ct p) -> p ct", p=P))
        gnb_sb = const.tile([P, CT], F32)
        nc.sync.dma_start(gnb_sb, gnb_p[:].rearrange("(ct p) -> p ct", p=P))
        bq_sb = const.tile([P, CT], F32)
        nc.sync.dma_start(bq_sb, bq_p[:].rearrange("(ct p) -> p ct", p=P))
        bp_sb = const.tile([P, CT], F32)
        nc.sync.dma_start(bp_sb, bp_p[:].rearrange("(ct p) -> p ct", p=P))

        # bf16 x streams in split across both HWDGE queue sets (SP and ACT)
        for sc in range(2 * NCH):
            _sl = slice(sc * CHUNK, (sc + 1) * CHUNK)
            eng = nc.sync if sc % 2 == 0 else nc.scalar
            eng.dma_start(x8_sb[:, :, _sl], x8r[:, :, _sl])
        # fp32 own-half (residual only) trickles in on the gpsimd queue;
        # first needed when group 0's proj output is assembled (~70us in)
        for sc in range(NCH):
            _sl = slice(sc * CHUNK, (sc + 1) * CHUNK)
            nc.gpsimd.dma_start(xa_sb[:, :, _sl], xar[:, :, _sl])

        wq_sb = const.tile([P, CT, C], F8)
        nc.sync.dma_start(wq_sb, wq_p[:].rearrange("(ci p) o -> p ci o", p=P))
        wk_sb = const.tile([P, CT, C], F8)
        nc.sync.dma_start(wk_sb, wk_p[:].rearrange("(ci p) o -> p ci o", p=P))
        wv_sb = const.tile([P, CT, C], F8)
        nc.sync.dma_start(wv_sb, wv_p[:].rearrange("(ci p) o -> p ci o", p=P))
        wp_sb = const.tile([P, CT, C], BF16)
        nc.sync.dma_start(wp_sb, wp_p[:].rearrange("(ci p) o -> p ci o", p=P))

        eps_t = const.tile([P, 1], F32)
        nc.vector.memset(eps_t, EPS)
        off_t = const.tile([P, 1], F32)
        nc.vector.memset(off_t, -OFF)
        ones16 = const.tile([P, 2, P], F8)
        nc.vector.memset(ones16, WS)
        junk = const.tile([P, CHUNK], BF16)
        nc.vector.memset(junk, 0.125)
        # block-diagonal group-averaging matrix over 64-channel groups
        ind = const.tile([P, P], F32)
        nc.vector.memset(ind, 0.0)
        nc.vector.memset(ind[0:64, 0:64], 1.0 / 64.0)
        nc.vector.memset(ind[64:128, 64:128], 1.0 / 64.0)

        # per-channel GN affine coefs (filled below)
        Acoef = const.tile([P, CT], F32)
        Bcoef = const.tile([P, CT], F32)

        # ---- PE warmup: dummy matmuls while x loads (HAM un-throttle) ----
        with tc.tile_pool(name="ps_warm", bufs=1, space="PSUM") as ps_w:
            warm_ps = ps_w.tile([P, CHUNK], F32)
            for _ in range(NWARM):
                nc.tensor.matmul(
                    warm_ps, lhsT=junk[:, 0:P], rhs=junk,
                    start=True, stop=True,
                )

        # ------- Phase 1a: GN statistics over both halves ----------
        with tc.tile_pool(name="p1a_s", bufs=1) as p1s, \
             tc.tile_pool(name="ps_g", bufs=1, space="PSUM") as ps_g:
            stats6 = p1s.tile([P, CT, 2 * NCH, 6], F32)
            for sc in range(2 * NCH):
                sl = slice(sc * CHUNK, (sc + 1) * CHUNK)
                for ct in range(CT):
                    nc.vector.bn_stats(
                        stats6[:, ct, sc, :], x8_sb[:, ct, sl]
                    )
            mv = p1s.tile([P, CT, 2], F32)
            for ct in range(CT):
                nc.vector.bn_aggr(mv[:, ct, :], stats6[:, ct, :, :])
            # per-channel moments: (mu, E[x^2] = var + mu^2)
            sm = p1s.tile([P, CT, 2], F32)
            nc.vector.tensor_mul(sm[:, :, 1], mv[:, :, 0], mv[:, :, 0])
            nc.vector.tensor_add(sm[:, :, 1], sm[:, :, 1], mv[:, :, 1])
            nc.vector.tensor_copy(sm[:, :, 0], mv[:, :, 0])
            # group moments, averaged over the 64 channels per group by ind
            gp = ps_g.tile([P, CT * 2], F32)
            nc.tensor.matmul(
                gp, lhsT=ind, rhs=sm.rearrange("p a b -> p (a b)"),
                start=True, stop=True,
            )
            gs = p1s.tile([P, CT, 2], F32)
            nc.vector.tensor_copy(gs.rearrange("p a b -> p (a b)"), gp)
            # var_g = E[x^2] - mu_g^2 ; rstd = 1/sqrt(var+eps)
            gvar = p1s.tile([P, CT], F32)
            nc.vector.tensor_mul(gvar, gs[:, :, 0], gs[:, :, 0])
            nc.vector.tensor_sub(gvar, gs[:, :, 1], gvar)
            gstd = p1s.tile([P, CT], F32)
            nc.scalar.activation(gstd, gvar, AF.Sqrt, bias=eps_t, scale=1.0)
            grstd = p1s.tile([P, CT], F32)
            nc.vector.reciprocal(grstd, gstd)
            # A = rstd * gn_w ; B = gn_b - mu * A
            nc.vector.tensor_mul(Acoef, grstd, gnw_sb)
            nc.vector.tensor_mul(Bcoef, gs[:, :, 0], Acoef)
            nc.vector.tensor_sub(Bcoef, gnb_sb, Bcoef)

        # ---------------- Phase 1b: h = GN(x) fp8; K, V, Q ----------------
        with tc.tile_pool(name="p1b_h", bufs=2) as pbh, \
             tc.tile_pool(name="ps_k", bufs=2, space="PSUM") as ps_k, \
             tc.tile_pool(name="ps_v", bufs=2, space="PSUM") as ps_v, \
             tc.tile_pool(name="ps_q", bufs=2, space="PSUM") as ps_q:

            for sc in range(2 * NCH):
                own = sc < NCH
                sl = slice((sc % NCH) * CHUNK, (sc % NCH + 1) * CHUNK)
                gsl = slice(sc * CHUNK, (sc + 1) * CHUNK)
                hc = pbh.tile([P, CT, CHUNK], F8, tag="hc")
                for ct in range(CT):
                    nc.vector.tensor_scalar(
                        hc[:, ct, :], x8_sb[:, ct, gsl],
                        Acoef[:, ct:ct + 1], Bcoef[:, ct:ct + 1],
                        mybir.AluOpType.mult, mybir.AluOpType.add,
                    )
                # K columns for this chunk (scalar engine does the copies)
                for co in range(CT):
                    ps = ps_k.tile([P, CHUNK], F32)
                    for t in range(2):
                        nc.tensor.matmul(
                            ps,
                            lhsT=wk_sb[:, 2 * t:2 * t + 2, co * P:(co + 1) * P],
                            rhs=hc[:, 2 * t:2 * t + 2, :],
                            start=(t == 0), stop=(t == 1), perf_mode=DR,
                        )
                    nc.scalar.copy(K_sb[:, co, gsl], ps)
                # V rows
                for mt in range(CHUNK // P):
                    ps = ps_v.tile([P, C], F32)
                    for t in range(2):
                        nc.tensor.matmul(
                            ps,
                            lhsT=hc[:, 2 * t:2 * t + 2, mt * P:(mt + 1) * P],
                            rhs=wv_sb[:, 2 * t:2 * t + 2, :],
                            start=(t == 0), stop=(t == 1), perf_mode=DR,
                        )
                    nc.vector.tensor_copy(
                        vT_sb[:, sc * (CHUNK // P) + mt, :], ps
                    )
                # Q (own half only)
                if own:
                    for co in range(CT):
                        ps = ps_q.tile([P, CHUNK], F32)
                        for t in range(2):
                            nc.tensor.matmul(
                                ps,
                                lhsT=wq_sb[:, 2 * t:2 * t + 2,
                                           co * P:(co + 1) * P],
                                rhs=hc[:, 2 * t:2 * t + 2, :],
                                start=(t == 0), stop=(t == 1), perf_mode=DR,
                            )
                        if has_bq:
                            nc.vector.tensor_scalar(
                                q_sb[:, co, sl], ps, bq_sb[:, co:co + 1],
                                None, mybir.AluOpType.add,
                            )
                        else:
                            nc.vector.tensor_copy(q_sb[:, co, sl], ps)

        # ---------------- Phase 2: attention + proj + residual ----------
        with tc.tile_pool(name="p2_pt", bufs=2) as ppt, \
             tc.tile_pool(name="p2_hg", bufs=2) as phg, \
             tc.tile_pool(name="p2_rd", bufs=2) as prd, \
             tc.tile_pool(name="p2_out", bufs=4) as pout, \
             tc.tile_pool(name="ps_s", bufs=2, space="PSUM") as ps_s, \
             tc.tile_pool(name="ps_pv", bufs=1, space="PSUM") as ps_pv, \
             tc.tile_pool(name="ps_od", bufs=1, space="PSUM") as ps_od:
            # ps_od is shared by the softmax denominator (PV region) and the
            # proj outputs (QK region) -- disjoint lifetimes, 2 banks total.

            def emit_proj(g, hg):
                """proj + residual + out DMA for group g (reads hg)."""
                gsl = slice(g * CHUNK, (g + 1) * CHUNK)
                for ot in range(CT):
                    ps = ps_od.tile([P, CHUNK], F32, tag="ps_o")
                    for cc in range(CT):
                        nc.tensor.matmul(
                            ps,
                            lhsT=wp_sb[:, cc, ot * P:(ot + 1) * P],
                            rhs=hg[:, cc, :],
                            start=(cc == 0), stop=(cc == CT - 1),
                        )
                    ot_sb = pout.tile([P, CHUNK], F32, tag="ot")
                    if has_bp:
                        nc.vector.tensor_scalar(
                            ot_sb, ps, bp_sb[:, ot:ot + 1], None,
                            mybir.AluOpType.add,
                        )
                        nc.vector.tensor_add(ot_sb, ot_sb, xa_sb[:, ot, gsl])
                    else:
                        nc.vector.tensor_add(ot_sb, ps, xa_sb[:, ot, gsl])
                    nc.sync.dma_start(outr[:, ot, gsl], ot_sb)

            hg_prev = None
            for g in range(NG):
                gsl = slice(g * CHUNK, (g + 1) * CHUNK)
                pT = ppt.tile([P, NT, CHUNK], F8, tag="pT")
                # scores (transposed) + exp, streaming per key tile
                for mt in range(NT):
                    ps = ps_s.tile([P, CHUNK], F32, tag="ps_s")
                    for t in range(2):
                        nc.tensor.matmul(
                            ps,
                            lhsT=K_sb[:, 2 * t:2 * t + 2, mt * P:(mt + 1) * P],
                            rhs=q_sb[:, 2 * t:2 * t + 2, gsl],
                            start=(t == 0), stop=(t == 1), perf_mode=DR,
                        )
                    nc.scalar.activation(
                        pT[:, mt, :], ps, AF.Exp, bias=off_t, scale=EXP_SCALE,
                    )
                    # interleave previous group's proj into the QK stream:
                    # its matmuls fill PE slack while ACT paces the exps
                    if hg_prev is not None and mt == 15:
                        emit_proj(g - 1, hg_prev)
                        hg_prev = None
                # PV + denominator, pairwise as exps complete
                d_ps = ps_od.tile([P, CHUNK], F32, tag="d")
                pvs = []
                for ct in range(CT):
                    pv_t = ps_pv.tile([P, CHUNK], F32, tag=f"pv{ct}")
                    pvs.append(pv_t)
                for j in range(NT // 2):
                    nc.tensor.matmul(
                        d_ps, lhsT=ones16, rhs=pT[:, 2 * j:2 * j + 2, :],
                        start=(j == 0), stop=(j == NT // 2 - 1), perf_mode=DR,
                    )
                    for ct in range(CT):
                        nc.tensor.matmul(
                            pvs[ct],
                            lhsT=vT_sb[:, 2 * j:2 * j + 2, ct * P:(ct + 1) * P],
                            rhs=pT[:, 2 * j:2 * j + 2, :],
                            start=(j == 0), stop=(j == NT // 2 - 1),
                            perf_mode=DR,
                        )
                rd = prd.tile([P, CHUNK], F32, tag="rd")
                nc.vector.reciprocal(rd, d_ps)
                hg = phg.tile([P, CT, CHUNK], BF16, tag="hg")
                for ct in range(CT):
                    nc.vector.tensor_mul(hg[:, ct, :], pvs[ct], rd)
                hg_prev = hg
            emit_proj(NG - 1, hg_prev)

    split_multi_waits(nc)
    return nc


_prog_cache: dict = {}


def _get_program(has_bq: bool, has_bp: bool) -> bass.Bass:
    key = (has_bq, has_bp)
    if key not in _prog_cache:
        _prog_cache[key] = build_program(has_bq, has_bp)
    return _prog_cache[key]


def make_in_maps(x, gn_w, gn_b, qkv_w, qkv_b, proj_w, proj_b):
    x = np.ascontiguousarray(np.asarray(x, dtype=np.float32))
    qkv_w = np.asarray(qkv_w, dtype=np.float32)
    qkv_b = np.asarray(qkv_b, dtype=np.float32)
    proj_w = np.asarray(proj_w, dtype=np.float32)
    proj_b = np.asarray(proj_b, dtype=np.float32)

    f8 = ml_dtypes.float8_e4m3fn
    wq8 = np.ascontiguousarray((qkv_w[0:C] * WS).T).astype(f8)
    wk8 = np.ascontiguousarray((qkv_w[C:2 * C] * WS).T).astype(f8)
    wv8 = np.ascontiguousarray((qkv_w[2 * C:3 * C] * WS).T).astype(f8)
    wp_bf = np.ascontiguousarray(proj_w.T).astype(ml_dtypes.bfloat16)
    bq16 = np.ascontiguousarray(qkv_b[0:C] * WS)
    # v-bias folds into proj bias: proj(h + bv) = proj(h) + proj_w @ bv
    # (softmax weights sum to 1). k-bias is softmax-invariant and dropped.
    bp = np.ascontiguousarray(proj_b + proj_w @ qkv_b[2 * C:3 * C])
    gn_w = np.ascontiguousarray(gn_w, dtype=np.float32)
    gn_b = np.ascontiguousarray(gn_b, dtype=np.float32)

    shared = {
        "wq8": wq8, "wk8": wk8, "wv8": wv8, "wp_bf": wp_bf,
        "bq16": bq16, "bp": bp, "gn_w": gn_w, "gn_b": gn_b,
    }
    in_maps = []
    x8_all = x.reshape(B, C, N).astype(ml_dtypes.bfloat16)
    for c in range(NCORES):
        b, v = divmod(c, 2)
        xb = x[b].reshape(C, N)
        x8b = x8_all[b]
        if v == 0:
            x8 = x8b
        else:
            x8 = np.concatenate([x8b[:, NQ:], x8b[:, :NQ]], axis=1)
        in_maps.append({
            "x8": np.ascontiguousarray(x8),
            "x_a": np.ascontiguousarray(xb[:, v * NQ:(v + 1) * NQ]),
            **shared,
        })
    has_bq = bool(np.any(bq16 != 0))
    has_bp = bool(np.any(bp != 0))
    return in_maps, has_bq, has_bp


def assemble_output(results) -> np.ndarray:
    out = np.empty((B, C, N), dtype=np.float32)
    for c in range(NCORES):
        b, v = divmod(c, 2)
        out[b, :, v * NQ:(v + 1) * NQ] = results[c]["out_q"]
    return out.reshape(B, C, H, W)


def run(inputs: dict, trace: bool = False):
    """Returns (output, BassKernelResults)."""
    in_maps, has_bq, has_bp = make_in_maps(**inputs)
    nc = _get_program(has_bq, has_bp)
    res = run_bass_kernel_spmd(nc, in_maps, list(range(NCORES)), trace=trace)
    return assemble_output(res.results), res


def kernel(**inputs) -> np.ndarray:
    out, _ = run(inputs)
    return out


# revision 17
# speedup vs baseline: 1.0491x; 1.0491x over previous
"""Trainium2 Bass kernel for an AttentionBlock (GroupNorm -> QKV 1x1 -> full
softmax attention over H*W tokens -> proj 1x1 -> residual).

Sharding: 8 cores = 4 batches x 2 query-halves, no collectives. Per core,
tokens are ordered [own half | other half]; attention is permutation-
invariant over keys, so K/V built in that order need no reshuffling.

Compute strategy (v2):
- fp8e4 DoubleRow matmuls (0.5 cyc/row) for K/V/Q/QK/PV; bf16 proj.
- Scores are computed TRANSPOSED (S^T[m,n] = sum_c K[c,m] Q[c,n]) so the
  exp() output is already in [key, query] layout and feeds the PV matmul
  directly -- no PE transposes at all. Row-sums (denominator) come from a
  ones-column DoubleRow matmul accumulated alongside PV.
- Weights are scaled by 16 host-side to center them in fp8e4 range; the
  resulting 16x factors cancel in softmax normalization (ones value = 16)
  and the 1/sqrt(C) score scale is folded into the exp() activation.
- x (both halves), Q, K, V all stay resident in SBUF: HBM traffic is just
  x in (8MB) + weights (~1.3MB) + out (4MB).
- Dummy bf16 matmuls keep the PE HAM-warm during the x-DMA/GN-stats
  preamble so real matmuls start at 2.4 GHz.

Self-contained: hardcodes shapes from the problem spec
(x: [4, 512, 64, 64] fp32).
"""

import sys

if "/opt/trn_rl_repo" not in sys.path:
    sys.path.insert(0, "/opt/trn_rl_repo")

from contextlib import ExitStack

import numpy as np
import ml_dtypes

import concourse.bass as bass
import concourse.tile as tile
from concourse import mybir
from concourse.bass_utils import run_bass_kernel_spmd

# Problem constants
B = 4
C = 512
H = 64
W = 64
N = H * W          # 4096 tokens
G = 8              # groupnorm groups
EPS = 1e-5
NCORES = 8
NQ = N // 2        # queries per core
P = 128
CT = C // P        # 4 channel tiles
NT = N // P        # 32 key tiles
CHUNK = 512        # n-chunk granularity
NCH = NQ // CHUNK  # 4 chunks per half
NG = NQ // CHUNK   # 4 query groups per core

WS = 16.0          # host-side fp8 weight scale
OFF = 2.5          # exp offset (S max is ~6.0 for this input)
EXP_SCALE = 1.0 / (np.sqrt(np.float32(C)) * WS * WS)

NWARM = 64         # PE warmup dummies during preamble

F32 = mybir.dt.float32
BF16 = mybir.dt.bfloat16
F8 = mybir.dt.float8e4
AF = mybir.ActivationFunctionType
DR = mybir.MatmulPerfMode.DoubleRow

MAX_WAITS_PER_INST = 1  # this walrus drop rejects >1 sync wait per inst


def split_multi_waits(nc: bass.Bass):
    """Walrus codegen here accepts at most one sync wait per instruction.
    Move excess waits onto freshly inserted same-engine NoOps directly
    before the offending instruction (waits just fire earlier)."""
    k = 0
    for fn in nc.m.functions:
        for bb in fn.blocks:
            insts = bb.instructions
            out = []
            changed = False
            for ins in insts:
                si = ins.sync_info
                if si is not None and len(si.on_wait) > MAX_WAITS_PER_INST:
                    waits = list(si.on_wait)
                    keep = waits[-MAX_WAITS_PER_INST:]
                    extra = waits[:-MAX_WAITS_PER_INST]
                    for i in range(0, len(extra), MAX_WAITS_PER_INST):
                        nop = mybir.InstNoOp(
                            name=f"{ins.name}_sw{k}", ins=[], outs=[]
                        )
                        k += 1
                        nop.engine = ins.engine
                        nop.sync_info = mybir.SyncInfo(
                            on_wait=extra[i:i + MAX_WAITS_PER_INST],
                            on_update=[],
                        )
                        out.append(nop)
                    ins.sync_info = mybir.SyncInfo(
                        on_wait=keep, on_update=list(si.on_update)
                    )
                    changed = True
                out.append(ins)
            if changed:
                bb.instructions = out


def build_program(has_bq: bool, has_bp: bool) -> bass.Bass:
    nc = bass.Bass()

    x8_p = nc.declare_dram_parameter("x8", [C, N], BF16, isOutput=False)
    x_a = nc.declare_dram_parameter("x_a", [C, NQ], F32, isOutput=False)
    wq_p = nc.declare_dram_parameter("wq8", [C, C], F8, isOutput=False)
    wk_p = nc.declare_dram_parameter("wk8", [C, C], F8, isOutput=False)
    wv_p = nc.declare_dram_parameter("wv8", [C, C], F8, isOutput=False)
    wp_p = nc.declare_dram_parameter("wp_bf", [C, C], BF16, isOutput=False)
    bq_p = nc.declare_dram_parameter("bq16", [C], F32, isOutput=False)
    bp_p = nc.declare_dram_parameter("bp", [C], F32, isOutput=False)
    gnw_p = nc.declare_dram_parameter("gn_w", [C], F32, isOutput=False)
    gnb_p = nc.declare_dram_parameter("gn_b", [C], F32, isOutput=False)
    out_q = nc.declare_dram_parameter("out_q", [C, NQ], F32, isOutput=True)

    # channel layout everywhere: c = ct*128 + p  (partition-inner)
    x8r = x8_p[:].rearrange("(ct p) n -> p ct n", p=P)
    xar = x_a[:].rearrange("(ct p) n -> p ct n", p=P)
    outr = out_q[:].rearrange("(ct p) n -> p ct n", p=P)

    with tile.TileContext(nc) as tc, ExitStack() as ctx:
        big = ctx.enter_context(tc.tile_pool(name="big", bufs=1))
        const = ctx.enter_context(tc.tile_pool(name="const", bufs=1))

        K_sb = big.tile([P, CT, N], F8)       # K^T: [c, m], 16x scaled
        vT_sb = big.tile([P, NT, C], F8)      # V: [m, c], 16x scaled
        q_sb = big.tile([P, CT, NQ], F8)      # Q^T: [c, n], 16x scaled
        x8_sb = big.tile([P, CT, N], BF16)    # bf16 x, both halves (stats/GN)
        xa_sb = big.tile([P, CT, NQ], F32)    # own half fp32 (residual only)

        # constants / weights: tiny DMAs first, then x, then big weights
        gnw_sb = const.tile([P, CT], F32)
        nc.sync.dma_start(gnw_sb, gnw_p[:].rearrange("(ct p) -> p ct", p=P))
        gnb_sb = const.tile([P, CT], F32)
        nc.sync.dma_start(gnb_sb, gnb_p[:].rearrange("(ct p) -> p ct", p=P))
        bq_sb = const.tile([P, CT], F32)
        nc.sync.dma_start(bq_sb, bq_p[:].rearrange("(ct p) -> p ct", p=P))
        bp_sb = const.tile([P, CT], F32)
        nc.sync.dma_start(bp_sb, bp_p[:].rearrange("(ct p) -> p ct", p=P))

        # bf16 x streams in split across both HWDGE queue sets (SP and ACT)
        for sc in range(2 * NCH):
            _sl = slice(sc * CHUNK, (sc + 1) * CHUNK)
            eng = nc.sync if sc % 2 == 0 else nc.scalar
            eng.dma_start(x8_sb[:, :, _sl], x8r[:, :, _sl])
        wq_sb = const.tile([P, CT, C], F8)
        nc.sync.dma_start(wq_sb, wq_p[:].rearrange("(ci p) o -> p ci o", p=P))
        wk_sb = const.tile([P, CT, C], F8)
        nc.sync.dma_start(wk_sb, wk_p[:].rearrange("(ci p) o -> p ci o", p=P))
        wv_sb = const.tile([P, CT, C], F8)
        nc.sync.dma_start(wv_sb, wv_p[:].rearrange("(ci p) o -> p ci o", p=P))
        wp_sb = const.tile([P, CT, C], BF16)
        nc.sync.dma_start(wp_sb, wp_p[:].rearrange("(ci p) o -> p ci o", p=P))
        # fp32 own-half (residual only): queued behind x8+weights on both
        # HWDGE queues, so it transfers after the critical preamble bytes.
        # First needed when group 0's proj output is assembled (~70us in).
        for sc in range(NCH):
            _sl = slice(sc * CHUNK, (sc + 1) * CHUNK)
            eng = nc.sync if sc % 2 == 0 else nc.scalar
            eng.dma_start(xa_sb[:, :, _sl], xar[:, :, _sl])

        eps_t = const.tile([P, 1], F32)
        nc.vector.memset(eps_t, EPS)
        off_t = const.tile([P, 1], F32)
        nc.vector.memset(off_t, -OFF)
        ones16 = const.tile([P, 2, P], F8)
        nc.vector.memset(ones16, WS)
        junk = const.tile([P, CHUNK], BF16)
        nc.vector.memset(junk, 0.125)
        # block-diagonal group-averaging matrix over 64-channel groups
        ind = const.tile([P, P], F32)
        nc.vector.memset(ind, 0.0)
        nc.vector.memset(ind[0:64, 0:64], 1.0 / 64.0)
        nc.vector.memset(ind[64:128, 64:128], 1.0 / 64.0)

        # per-channel GN affine coefs (filled below)
        Acoef = const.tile([P, CT], F32)
        Bcoef = const.tile([P, CT], F32)

        # ---- PE warmup: dummy matmuls while x loads (HAM un-throttle) ----
        with tc.tile_pool(name="ps_warm", bufs=1, space="PSUM") as ps_w:
            warm_ps = ps_w.tile([P, CHUNK], F32)
            for _ in range(NWARM):
                nc.tensor.matmul(
                    warm_ps, lhsT=junk[:, 0:P], rhs=junk,
                    start=True, stop=True,
                )

        # ------- Phase 1a: GN statistics over both halves ----------
        with tc.tile_pool(name="p1a_s", bufs=1) as p1s, \
             tc.tile_pool(name="ps_g", bufs=1, space="PSUM") as ps_g:
            stats6 = p1s.tile([P, CT, 2 * NCH, 6], F32)
            for sc in range(2 * NCH):
                sl = slice(sc * CHUNK, (sc + 1) * CHUNK)
                for ct in range(CT):
                    nc.vector.bn_stats(
                        stats6[:, ct, sc, :], x8_sb[:, ct, sl]
                    )
            mv = p1s.tile([P, CT, 2], F32)
            for ct in range(CT):
                nc.vector.bn_aggr(mv[:, ct, :], stats6[:, ct, :, :])
            # per-channel moments: (mu, E[x^2] = var + mu^2)
            sm = p1s.tile([P, CT, 2], F32)
            nc.vector.tensor_mul(sm[:, :, 1], mv[:, :, 0], mv[:, :, 0])
            nc.vector.tensor_add(sm[:, :, 1], sm[:, :, 1], mv[:, :, 1])
            nc.vector.tensor_copy(sm[:, :, 0], mv[:, :, 0])
            # group moments, averaged over the 64 channels per group by ind
            gp = ps_g.tile([P, CT * 2], F32)
            nc.tensor.matmul(
                gp, lhsT=ind, rhs=sm.rearrange("p a b -> p (a b)"),
                start=True, stop=True,
            )
            gs = p1s.tile([P, CT, 2], F32)
            nc.vector.tensor_copy(gs.rearrange("p a b -> p (a b)"), gp)
            # var_g = E[x^2] - mu_g^2 ; rstd = 1/sqrt(var+eps)
            gvar = p1s.tile([P, CT], F32)
            nc.vector.tensor_mul(gvar, gs[:, :, 0], gs[:, :, 0])
            nc.vector.tensor_sub(gvar, gs[:, :, 1], gvar)
            gstd = p1s.tile([P, CT], F32)
            nc.scalar.activation(gstd, gvar, AF.Sqrt, bias=eps_t, scale=1.0)
            grstd = p1s.tile([P, CT], F32)
            nc.vector.reciprocal(grstd, gstd)
            # A = rstd * gn_w ; B = gn_b - mu * A
            nc.vector.tensor_mul(Acoef, grstd, gnw_sb)
            nc.vector.tensor_mul(Bcoef, gs[:, :, 0], Acoef)
            nc.vector.tensor_sub(Bcoef, gnb_sb, Bcoef)

        # ---------------- Phase 1b: h = GN(x) fp8; K, V, Q ----------------
        with tc.tile_pool(name="p1b_h", bufs=2) as pbh, \
             tc.tile_pool(name="ps_k", bufs=2, space="PSUM") as ps_k, \
             tc.tile_pool(name="ps_v", bufs=2, space="PSUM") as ps_v, \
             tc.tile_pool(name="ps_q", bufs=2, space="PSUM") as ps_q:

            for sc in range(2 * NCH):
                own = sc < NCH
                sl = slice((sc % NCH) * CHUNK, (sc % NCH + 1) * CHUNK)
                gsl = slice(sc * CHUNK, (sc + 1) * CHUNK)
                hc = pbh.tile([P, CT, CHUNK], F8, tag="hc")
                for ct in range(CT):
                    nc.vector.tensor_scalar(
                        hc[:, ct, :], x8_sb[:, ct, gsl],
                        Acoef[:, ct:ct + 1], Bcoef[:, ct:ct + 1],
                        mybir.AluOpType.mult, mybir.AluOpType.add,
                    )
                # K columns for this chunk (scalar engine does the copies)
                for co in range(CT):
                    ps = ps_k.tile([P, CHUNK], F32)
                    for t in range(2):
                        nc.tensor.matmul(
                            ps,
                            lhsT=wk_sb[:, 2 * t:2 * t + 2, co * P:(co + 1) * P],
                            rhs=hc[:, 2 * t:2 * t + 2, :],
                            start=(t == 0), stop=(t == 1), perf_mode=DR,
                        )
                    nc.scalar.copy(K_sb[:, co, gsl], ps)
                # V rows
                for mt in range(CHUNK // P):
                    ps = ps_v.tile([P, C], F32)
                    for t in range(2):
                        nc.tensor.matmul(
                            ps,
                            lhsT=hc[:, 2 * t:2 * t + 2, mt * P:(mt + 1) * P],
                            rhs=wv_sb[:, 2 * t:2 * t + 2, :],
                            start=(t == 0), stop=(t == 1), perf_mode=DR,
                        )
                    nc.vector.tensor_copy(
                        vT_sb[:, sc * (CHUNK // P) + mt, :], ps
                    )
                # Q (own half only)
                if own:
                    for co in range(CT):
                        ps = ps_q.tile([P, CHUNK], F32)
                        for t in range(2):
                            nc.tensor.matmul(
                                ps,
                                lhsT=wq_sb[:, 2 * t:2 * t + 2,
                                           co * P:(co + 1) * P],
                                rhs=hc[:, 2 * t:2 * t + 2, :],
                                start=(t == 0), stop=(t == 1), perf_mode=DR,
                            )
                        if has_bq:
                            nc.vector.tensor_scalar(
                                q_sb[:, co, sl], ps, bq_sb[:, co:co + 1],
                                None, mybir.AluOpType.add,
                            )
                        else:
                            nc.vector.tensor_copy(q_sb[:, co, sl], ps)

        # ---------------- Phase 2: attention + proj + residual ----------
        with tc.tile_pool(name="p2_pt", bufs=2) as ppt, \
             tc.tile_pool(name="p2_hg", bufs=2) as phg, \
             tc.tile_pool(name="p2_rd", bufs=2) as prd, \
             tc.tile_pool(name="p2_out", bufs=4) as pout, \
             tc.tile_pool(name="ps_s", bufs=2, space="PSUM") as ps_s, \
             tc.tile_pool(name="ps_pv", bufs=1, space="PSUM") as ps_pv, \
             tc.tile_pool(name="ps_od", bufs=1, space="PSUM") as ps_od:
            # ps_od is shared by the softmax denominator (PV region) and the
            # proj outputs (QK region) -- disjoint lifetimes, 2 banks total.

            def emit_proj(g, hg):
                """proj + residual + out DMA for group g (reads hg)."""
                gsl = slice(g * CHUNK, (g + 1) * CHUNK)
                for ot in range(CT):
                    ps = ps_od.tile([P, CHUNK], F32, tag="ps_o")
                    for cc in range(CT):
                        nc.tensor.matmul(
                            ps,
                            lhsT=wp_sb[:, cc, ot * P:(ot + 1) * P],
                            rhs=hg[:, cc, :],
                            start=(cc == 0), stop=(cc == CT - 1),
                        )
                    ot_sb = pout.tile([P, CHUNK], F32, tag="ot")
                    if has_bp:
                        nc.vector.tensor_scalar(
                            ot_sb, ps, bp_sb[:, ot:ot + 1], None,
                            mybir.AluOpType.add,
                        )
                        nc.vector.tensor_add(ot_sb, ot_sb, xa_sb[:, ot, gsl])
                    else:
                        nc.vector.tensor_add(ot_sb, ps, xa_sb[:, ot, gsl])
                    nc.sync.dma_start(outr[:, ot, gsl], ot_sb)

            hg_prev = None
            for g in range(NG):
                gsl = slice(g * CHUNK, (g + 1) * CHUNK)
                pT = ppt.tile([P, NT, CHUNK], F8, tag="pT")
                # scores (transposed) + exp, streaming per key tile
                for mt in range(NT):
                    ps = ps_s.tile([P, CHUNK], F32, tag="ps_s")
                    for t in range(2):
                        nc.tensor.matmul(
                            ps,
                            lhsT=K_sb[:, 2 * t:2 * t + 2, mt * P:(mt + 1) * P],
                            rhs=q_sb[:, 2 * t:2 * t + 2, gsl],
                            start=(t == 0), stop=(t == 1), perf_mode=DR,
                        )
                    nc.scalar.activation(
                        pT[:, mt, :], ps, AF.Exp, bias=off_t, scale=EXP_SCALE,
                    )
                    # interleave previous group's proj into the QK stream:
                    # its matmuls fill PE slack while ACT paces the exps
                    if hg_prev is not None and mt == 15:
                        emit_proj(g - 1, hg_prev)
                        hg_prev = None
                # PV + denominator, pairwise as exps complete
                d_ps = ps_od.tile([P, CHUNK], F32, tag="d")
                pvs = []
                for ct in range(CT):
                    pv_t = ps_pv.tile([P, CHUNK], F32, tag=f"pv{ct}")
                    pvs.append(pv_t)
                for j in range(NT // 2):
                    nc.tensor.matmul(
                        d_ps, lhsT=ones16, rhs=pT[:, 2 * j:2 * j + 2, :],
                        start=(j == 0), stop=(j == NT // 2 - 1), perf_mode=DR,
                    )
                    for ct in range(CT):
                        nc.tensor.matmul(
                            pvs[ct],
                            lhsT=vT_sb[:, 2 * j:2 * j + 2, ct * P:(ct + 1) * P],
                            rhs=pT[:, 2 * j:2 * j + 2, :],
                            start=(j == 0), stop=(j == NT // 2 - 1),
                            perf_mode=DR,
                        )
                rd = prd.tile([P, CHUNK], F32, tag="rd")
                nc.vector.reciprocal(rd, d_ps)
                hg = phg.tile([P, CT, CHUNK], BF16, tag="hg")
                for ct in range(CT):
                    nc.vector.tensor_mul(hg[:, ct, :], pvs[ct], rd)
                hg_prev = hg
            emit_proj(NG - 1, hg_prev)

    split_multi_waits(nc)
    return nc


_prog_cache: dict = {}


def _get_program(has_bq: bool, has_bp: bool) -> bass.Bass:
    key = (has_bq, has_bp)
    if key not in _prog_cache:
        _prog_cache[key] = build_program(has_bq, has_bp)
    return _prog_cache[key]


def make_in_maps(x, gn_w, gn_b, qkv_w, qkv_b, proj_w, proj_b):
    x = np.ascontiguousarray(np.asarray(x, dtype=np.float32))
    qkv_w = np.asarray(qkv_w, dtype=np.float32)
    qkv_b = np.asarray(qkv_b, dtype=np.float32)
    proj_w = np.asarray(proj_w, dtype=np.float32)
    proj_b = np.asarray(proj_b, dtype=np.float32)

    f8 = ml_dtypes.float8_e4m3fn
    wq8 = np.ascontiguousarray((qkv_w[0:C] * WS).T).astype(f8)
    wk8 = np.ascontiguousarray((qkv_w[C:2 * C] * WS).T).astype(f8)
    wv8 = np.ascontiguousarray((qkv_w[2 * C:3 * C] * WS).T).astype(f8)
    wp_bf = np.ascontiguousarray(proj_w.T).astype(ml_dtypes.bfloat16)
    bq16 = np.ascontiguousarray(qkv_b[0:C] * WS)
    # v-bias folds into proj bias: proj(h + bv) = proj(h) + proj_w @ bv
    # (softmax weights sum to 1). k-bias is softmax-invariant and dropped.
    bp = np.ascontiguousarray(proj_b + proj_w @ qkv_b[2 * C:3 * C])
    gn_w = np.ascontiguousarray(gn_w, dtype=np.float32)
    gn_b = np.ascontiguousarray(gn_b, dtype=np.float32)

    shared = {
        "wq8": wq8, "wk8": wk8, "wv8": wv8, "wp_bf": wp_bf,
        "bq16": bq16, "bp": bp, "gn_w": gn_w, "gn_b": gn_b,
    }
    in_maps = []
    x8_all = x.reshape(B, C, N).astype(ml_dtypes.bfloat16)
    for c in range(NCORES):
        b, v = divmod(c, 2)
        xb = x[b].reshape(C, N)
        x8b = x8_all[b]
        if v == 0:
            x8 = x8b
        else:
            x8 = np.concatenate([x8b[:, NQ:], x8b[:, :NQ]], axis=1)
        in_maps.append({
            "x8": np.ascontiguousarray(x8),
            "x_a": np.ascontiguousarray(xb[:, v * NQ:(v + 1) * NQ]),
            **shared,
        })
    has_bq = bool(np.any(bq16 != 0))
    has_bp = bool(np.any(bp != 0))
    return in_maps, has_bq, has_bp


def assemble_output(results) -> np.ndarray:
    out = np.empty((B, C, N), dtype=np.float32)
    for c in range(NCORES):
        b, v = divmod(c, 2)
        out[b, :, v * NQ:(v + 1) * NQ] = results[c]["out_q"]
    return out.reshape(B, C, H, W)


def run(inputs: dict, trace: bool = False):
    """Returns (output, BassKernelResults)."""
    in_maps, has_bq, has_bp = make_in_maps(**inputs)
    nc = _get_program(has_bq, has_bp)
    res = run_bass_kernel_spmd(nc, in_maps, list(range(NCORES)), trace=trace)
    return assemble_output(res.results), res


def kernel(**inputs) -> np.ndarray:
    out, _ = run(inputs)
    return out
